# revision 61
# baseline (speedup 1.0000x reference)
"""Trainium2 Bass kernel for the Kruskal (CP/Tucker) linear layer.

Math: the reference reconstructs W (4096x4096) from a rank-16 CP core and
Tucker factors, then computes y = x @ W.T + bias.  Because the 6D core is a
CP (Kruskal) tensor of rank 16, W itself is exactly rank 16:

    W = g_out @ g_in.T
    g_in[def, r]  = (f3@c3)[d,r] * (f4@c4)[e,r] * (f5@c5)[f,r]   (4096 x 16)
    g_out[abc, r] = (f0@c0)[a,r] * (f1@c1)[b,r] * (f2@c2)[c,r]   (4096 x 16)

so  y = (x @ g_in) @ g_out.T + bias.  The device kernel computes the two
x-dependent projections; the tiny factor-only products (g_in/g_out, ~100
KFLOP) are prepared on the host.

Sharding: data-parallel over the batch (4096 rows -> 8 cores x 512). No
collectives.  The host ships each core its x slice PRE-TRANSPOSED to
feature-major bf16, split into two BATCH HALVES (256 rows each): all 32
k-tiles of half 0 load first, so stage 1 / stage 2 / y stores for half 0
run while half 1 is still loading.  This overlaps the ~22us y write-back
with the x load instead of serializing them.  Per core, per half:
  1. HWDGE loads of x^T k-groups (128, kt, 256) bf16 (gin rides inside the
     first load; gout+bias+t^T-init ride as one early const load)
  2. stage 1: 32 accumulating matmuls  t^T(16,256) += g_in_kt.T @ x^T_kt
     as one PSUM start/stop group per load (groups start as their x lands)
  3. DVE copy t^T half -> SBUF bf16 (ones-row for the bias pre-loaded)
  4. stage 2: 16 bf16 matmuls (2 batch tiles x 8 col tiles, N=512)
     y = [t,1] @ [g_out.T; bias]
  5. DVE/ACT copy PSUM->SBUF per 1024 cols, DMA y fp32 out per 2048 cols
"""

import numpy as np
import ml_dtypes

N_CORES = 8
BATCH = 4096
D = 4096          # in/out features (16*16*16)
R = 16            # CP rank
P = 128           # partitions
NB = BATCH // N_CORES   # 512 batch rows per core
HB = NB // 2            # 256 batch rows per half
BT = NB // P            # 4 batch tiles per core
KT = D // P             # 32 feature k-tiles
NT = 512                # output column tile (PSUM bank / max moving size)
JT = D // NT            # 8 output column tiles
GS = [8, 12, 12]        # k-tiles per DMA load group within a half (sums to
                        # KT).  Few, fat loads: the first matmul can't start
                        # before ~13-18us anyway (fixed preamble + DMA-engine
                        # semaphore backlog), by which time most x has
                        # landed — fine-grained groups only stall the PE out
                        # of its fast p-state

_PROGRAM = None


def _build_program():
    import concourse.tile as tile
    from concourse import bacc, mybir

    nc = bacc.Bacc(
        "TRN2",
        target_bir_lowering=False,
        debug=False,
        enable_asserts=False,
        num_devices=N_CORES,
    )
    # x^T per half, feature-major k-tile-major bf16, host-packed:
    # xh[h][p, kt*HB + b] = x[h*HB + b, kt*128 + p] (core-local batch index).
    # Half 0's first group additionally carries gin in front of its columns.
    xg0_d = nc.dram_tensor(
        "xg0c", (P, KT * R + GS[0] * HB), mybir.dt.bfloat16, kind="ExternalInput"
    )
    xr0_d = nc.dram_tensor(
        "xr0c", (P, (KT - GS[0]) * HB), mybir.dt.bfloat16, kind="ExternalInput"
    )
    xr1_d = nc.dram_tensor(
        "xr1c", (P, KT * HB), mybir.dt.bfloat16, kind="ExternalInput"
    )
    # gout packed with the t^T init image (rows 0..15 zeros, row 16 ones for
    # the bias): [:, :D] = [g_out.T; bias], [:, D:] = t^T init
    gouta_d = nc.dram_tensor(
        "gouta", (R + 1, D + NB), mybir.dt.bfloat16, kind="ExternalInput"
    )
    y_d = nc.dram_tensor("yc", (NB, D), mybir.dt.float32, kind="ExternalOutput")

    with tile.TileContext(nc) as tc:
        with (
            tc.tile_pool(name="const", bufs=1) as constp,
            tc.tile_pool(name="xT", bufs=2 * len(GS)) as xTp,
            tc.tile_pool(name="ysb", bufs=4) as ysbp,
            tc.tile_pool(name="tpsum", bufs=1, space="PSUM") as tpsump,
            tc.tile_pool(name="ypsum", bufs=3, space="PSUM") as ypsump,
        ):
            # half-0 group 0 (with gin packed in front) first on scalar, the
            # const pack early on sync, then the remaining groups alternate
            # queues in consumption order
            xg0 = xTp.tile([P, KT * R + GS[0] * HB], mybir.dt.bfloat16)
            nc.scalar.dma_start(xg0[:], xg0_d.ap())
            gin_sb = xg0[:, 0 : KT * R]

            gouta_sb = constp.tile([R + 1, D + NB], mybir.dt.bfloat16)
            nc.sync.dma_start(gouta_sb[:], gouta_d.ap())
            gout_sb = gouta_sb[:, 0:D]
            tT_sb = gouta_sb[:, D : D + NB]

            # remaining loads: half 0 groups 1.., then half 1 groups 0..
            xT_sb = [[xg0[:, KT * R :]], []]
            qi = 0
            for h, src in ((0, xr0_d), (1, xr1_d)):
                off = 0
                for ng in range(1 if h == 0 else 0, len(GS)):
                    xt = xTp.tile([P, GS[ng] * HB], mybir.dt.bfloat16)
                    eng = (nc.sync, nc.scalar)[qi % 2]
                    qi += 1
                    eng.dma_start(
                        xt[:], src.ap()[:, off * HB : (off + GS[ng]) * HB]
                    )
                    xT_sb[h].append(xt)
                    off += GS[ng]

            # stage 1 per half: one PSUM start/stop group per x load so each
            # group's matmuls fire as soon as its x lands (a single large
            # accumulation group would wait for the whole half).  PSUM
            # accumulation is per-write on HW, so chaining groups with
            # start=False is exact.  Halves write disjoint column ranges of
            # one PSUM tile.
            tT_ps = tpsump.tile([R, NB], mybir.dt.float32)
            cp = 0
            y_jobs = []
            for h in range(2):
                kt = 0
                for ng in range(len(GS)):
                    for g in range(GS[ng]):
                        nc.tensor.matmul(
                            tT_ps[:, h * HB : (h + 1) * HB],
                            lhsT=gin_sb[:, kt * R : (kt + 1) * R],
                            rhs=xT_sb[h][ng][:, g * HB : (g + 1) * HB],
                            start=(kt == 0),
                            stop=(g == GS[ng] - 1),
                            skip_group_check=True,
                        )
                        kt += 1
                # t^T half: rows 0..15 cast to bf16 (row 16 ones pre-loaded)
                nc.vector.tensor_copy(
                    tT_sb[0:R, h * HB : (h + 1) * HB],
                    tT_ps[:, h * HB : (h + 1) * HB],
                )

                # stage 2 for this half: 2 batch tiles x 8 col tiles (N=512);
                # jt pairs share a 2-bank PSUM tile, one 1024-col copy per
                # pair alternating DVE/ACT, y out per 2048 cols on sync
                for i in range(2):
                    bt = h * 2 + i
                    y_sb = ysbp.tile([P, D], mybir.dt.float32)
                    for jp in range(JT // 2):
                        y_ps = ypsump.tile([P, 2 * NT], mybir.dt.float32)
                        for u in range(2):
                            jt = jp * 2 + u
                            nc.tensor.matmul(
                                y_ps[:, u * NT : (u + 1) * NT],
                                lhsT=tT_sb[:, bt * P : (bt + 1) * P],
                                rhs=gout_sb[:, jt * NT : (jt + 1) * NT],
                            )
                        dst = y_sb[:, jp * 2 * NT : (jp + 1) * 2 * NT]
                        if cp % 2 == 0:
                            nc.vector.tensor_copy(dst, y_ps[:])
                        else:
                            nc.scalar.copy(dst, y_ps[:])
                        cp += 1
                        if jp % 2 == 1:
                            yh = jp // 2
                            nc.sync.dma_start(
                                y_d.ap()[
                                    bt * P : (bt + 1) * P,
                                    yh * (D // 2) : (yh + 1) * (D // 2),
                                ],
                                y_sb[:, yh * (D // 2) : (yh + 1) * (D // 2)],
                            )

    nc.compile()
    return nc


def _get_program():
    global _PROGRAM
    if _PROGRAM is None:
        _PROGRAM = _build_program()
    return _PROGRAM


def _host_factors(inputs):
    """Build g_in (SBUF layout) and [g_out.T; bias | t^T-init] (bf16)."""
    c = [np.asarray(inputs[f"c{i}"], dtype=np.float64) for i in range(6)]
    f = [np.asarray(inputs[f"f{i}"], dtype=np.float64) for i in range(6)]
    bias = np.asarray(inputs["bias"], dtype=np.float32)
    h = [f[i] @ c[i] for i in range(6)]  # (16,16) each
    g_out = (
        h[0][:, None, None, :] * h[1][None, :, None, :] * h[2][None, None, :, :]
    ).reshape(D, R)
    g_in = (
        h[3][:, None, None, :] * h[4][None, :, None, :] * h[5][None, None, :, :]
    ).reshape(D, R)
    # gin SBUF layout: gin_l[p, kt*R + r] = g_in[kt*128 + p, r]
    gin_l = np.ascontiguousarray(
        g_in.reshape(KT, P, R).transpose(1, 0, 2).reshape(P, KT * R)
    ).astype(ml_dtypes.bfloat16)
    goutT = np.concatenate(
        [g_out.T.astype(np.float32), bias[None, :]], axis=0
    ).astype(ml_dtypes.bfloat16)  # (17, 4096)
    gouta = np.zeros((R + 1, D + NB), dtype=ml_dtypes.bfloat16)
    gouta[:, 0:D] = goutT
    gouta[R, D:] = 1.0
    return gin_l, gouta


# test-harness hooks (unused in graded path)
TRACE = False
LAST_RESULTS = None


def kernel(**inputs):
    from concourse.bass_utils import run_bass_kernel_spmd

    global LAST_RESULTS
    x = np.asarray(inputs["x"], dtype=np.float32)
    # host-side: cast to bf16, transpose to feature-major, split each core's
    # rows into two halves, k-tile-major within a half:
    # xh[ci, h][p, kt*HB + b] = x[ci*NB + h*HB + b, kt*128 + p]
    xb = x.astype(ml_dtypes.bfloat16)  # (BATCH, D)
    xh = np.ascontiguousarray(
        xb.reshape(N_CORES, 2, HB, KT, P).transpose(0, 1, 4, 3, 2)
    ).reshape(N_CORES, 2, P, KT * HB)
    gin_l, gouta = _host_factors(inputs)
    # half-0 group 0 with gin packed in front of its first GS[0] k-tiles
    g0w = GS[0] * HB
    xg0 = np.empty((N_CORES, P, KT * R + g0w), dtype=ml_dtypes.bfloat16)
    xg0[:, :, 0 : KT * R] = gin_l[None]
    xg0[:, :, KT * R :] = xh[:, 0, :, 0:g0w]
    nc = _get_program()
    in_maps = [
        {
            "xg0c": xg0[ci],
            "xr0c": np.ascontiguousarray(xh[ci, 0, :, g0w:]),
            "xr1c": xh[ci, 1],
            "gouta": gouta,
        }
        for ci in range(N_CORES)
    ]
    res = run_bass_kernel_spmd(
        nc, in_maps, core_ids=list(range(N_CORES)), trace=TRACE
    )
    LAST_RESULTS = res
    y = np.concatenate([r["yc"] for r in res.results], axis=0)
    return np.ascontiguousarray(y.astype(np.float32))


if __name__ == "__main__":
    # quick smoke test with random data
    rng = np.random.default_rng(0)
    ins = {"x": rng.normal(size=(BATCH, D)).astype(np.float32)}
    for i in range(6):
        ins[f"c{i}"] = (rng.normal(size=(8, 16)) * 0.1).astype(np.float32)
        ins[f"f{i}"] = (rng.normal(size=(16, 8)) * 0.1).astype(np.float32)
    ins["bias"] = np.zeros(D, dtype=np.float32)
    y = kernel(**ins)
    print("y", y.shape, y.dtype)
